# revision 2
# baseline (speedup 1.0000x reference)
"""Self-contained E8 lattice quantizer for Trainium2 (8 NeuronCores), v2.

kernel(x) -> nearest-E8-point of each row of x [8388608, 8] f32.

Algorithm (per group of 8 contiguous elements = one row):
  t1 = x + MAGIC ; f1 = t1 - MAGIC          (round-half-even via magic const)
  d1 = x - f1
  E  = (d1.bits & 0x7FFFFFF8) | (7 - idx)   (abs-bits + tie-break index)
  group stats via strided pairwise step + reduce4:
    M = max(E)  -> argmax|d1| encode        (coset-1 nudge target)
    Mn = min(E) -> argmin|d1| encode        (coset-2 nudge target)
    A1s = sum(E as f32) ~= sum|d1|
    Pe = sum((t1&1) + 32*neg)               (parity of sum f1, count of neg)
  small [128,R] chain: parities via MAGIC-LSB, coset decision
    cw <=> par1*(2M-1) + 2 + 2*par2*Mn < A1s   (q2 < q1)
    Tsel = cw ? Mn*(2*par2-1) : M*(2*par1-1)   (negative => never matches)
  mt = (E == bc(Tsel)) as bf16; sigma = (mt ^ bc(cw*0x8000)) + bc(cw*0.5)
  out = f1 + (sigma ^ sign16(d1))           (bf16 bit-xor applies sign of d1)

Engine split balances ACT (rounding) / Pool / DVE per the v1 cost model.
Sharding: rows split evenly across 8 cores (data parallel, no comms).
"""
import numpy as np
import concourse.bass as bass
import concourse.mybir as mybir
from concourse.tile import TileContext
from concourse.bass_utils import run_bass_kernel_spmd

AL = mybir.AluOpType
AF = mybir.ActivationFunctionType
F32 = mybir.dt.float32
I32 = mybir.dt.int32
U16 = mybir.dt.uint16
BF16 = mybir.dt.bfloat16
MAGIC = float(np.float32(12582912.0))  # 1.5 * 2^23

N_ROWS_FULL = 8388608
DIM = 8
NCORES = 8
ROWS = N_ROWS_FULL // NCORES
F = 1024  # free-dim elems per partition per tile


def _split_multiwaits(nc):
    """This walrus build rejects >1 sem wait per instruction: hoist extras
    onto standalone nops inserted immediately before."""
    n = 0
    for f in nc.m.functions:
        for bb in f.blocks:
            newlist = []
            for ins in bb.instructions:
                si = getattr(ins, "sync_info", None)
                if si is not None and si.on_wait is not None and len(si.on_wait) > 1:
                    waits = list(si.on_wait)
                    for w in waits[:-1]:
                        nop = mybir.InstNoOp(name=f"I-mwfix-{n}", ins=[], outs=[])
                        n += 1
                        nop.engine = ins.engine
                        nop.sync_info = mybir.SyncInfo(on_wait=[w], on_update=[])
                        newlist.append(nop)
                    si.on_wait = [waits[-1]]
                newlist.append(ins)
            bb.instructions = newlist
    return n


def _g3(ap, c=8):
    return ap.rearrange("p (r c) -> p r c", c=c)


def _bc(ap_2d, c=8):
    p, r = ap_2d.shape
    return ap_2d.unsqueeze(2).broadcast_to((p, r, c))


def build_nc(rows=ROWS, f=F, num_devices=NCORES, fix_multiwaits=True):
    elems = rows * DIM
    assert elems % (128 * f) == 0
    ntiles = elems // (128 * f)
    R = f // 8

    nc = bass.Bass("TRN2", num_devices=num_devices, debug=False)
    x = nc.dram_tensor("x", [rows, DIM], F32, kind="ExternalInput")
    y = nc.dram_tensor("y", [rows, DIM], F32, kind="ExternalOutput")
    xt = x[:].flatten().rearrange("(t p f) -> t p f", p=128, f=f)
    yt = y[:].flatten().rearrange("(t p f) -> t p f", p=128, f=f)

    with TileContext(nc) as tc:
        with tc.tile_pool(name="cst", bufs=1) as cst, \
             tc.tile_pool(name="io", bufs=4) as io, \
             tc.tile_pool(name="wk", bufs=4) as wk, \
             tc.tile_pool(name="h16", bufs=4) as h16, \
             tc.tile_pool(name="g4", bufs=2) as g4, \
             tc.tile_pool(name="gr", bufs=3) as gr, \
             tc.tile_pool(name="ps", bufs=2, space="PSUM") as ps:

            # constant: (7 - idx%8) repeating along free dim
            idxf = cst.tile([128, f], I32)
            nc.gpsimd.iota(idxf[:], pattern=[[0, R], [1, 8]], base=0,
                           channel_multiplier=0)
            idxr = cst.tile([128, f], I32)
            nc.vector.tensor_scalar(idxr[:], idxf[:], -1, 7, AL.mult, AL.add)
            ii = cst.tile([128, 128], I32)
            nc.gpsimd.iota(ii[:], pattern=[[0, 128]], base=0, channel_multiplier=1)
            jj = cst.tile([128, 128], I32)
            nc.gpsimd.iota(jj[:], pattern=[[1, 128]], base=0, channel_multiplier=0)
            ident = cst.tile([128, 128], BF16)
            nc.vector.tensor_tensor(ident[:], ii[:], jj[:], AL.is_equal)
            mskA = cst.tile([128, 1], I32)
            nc.vector.memset(mskA[:], 0x7FFFFFF8)

            V, P = nc.vector, nc.gpsimd

            for t in range(ntiles):
                xv = io.tile([128, f], F32, tag="xv")
                nc.sync.dma_start(xv[:], xt[t])

                # rounding (ACT)
                t1 = wk.tile([128, f], F32, tag="t1")
                nc.scalar.activation(t1[:], xv[:], AF.Copy, bias=MAGIC)
                f1 = h16.tile([128, f], BF16, tag="f1")
                nc.scalar.activation(f1[:], t1[:], AF.Copy, bias=-MAGIC)

                # d1 (Pool)
                d1 = wk.tile([128, f], F32, tag="d1")
                P.tensor_tensor(d1[:], xv[:], f1[:], AL.subtract)
                d1i = d1[:].bitcast(I32)

                # E encode (DVE): (d1 & 0x7FFFFFF8) | idxr
                E = wk.tile([128, f], I32, tag="E")
                V.scalar_tensor_tensor(E[:], d1i, mskA[:, 0:1], idxr[:],
                                       AL.bitwise_and, AL.bitwise_or)
                Ef = E[:].bitcast(F32)

                # shalf = +-0.5 by sign of d1 (Pool, arith)
                shalf = h16.tile([128, f], BF16, tag="shalf")
                P.tensor_scalar(shalf[:], d1[:], 0.0, 0.5, AL.is_ge, AL.subtract)
                # podd = t1 & 1 (DVE, bitVec i32)
                podd = g4.tile([128, f], I32, tag="podd")
                V.tensor_scalar(podd[:], t1[:].bitcast(I32), 1, None,
                                AL.bitwise_and)

                # --- group reductions ---
                MMn = gr.tile([128, 2 * R], F32, tag="MMn")

                def gred4(src_ap, op, tag, out_ap, eng):
                    s4 = g4.tile([128, f // 2], F32, tag=tag + "4")
                    a = _g3(src_ap)
                    eng.tensor_tensor(_g3(s4[:], 4), a[:, :, 0:4],
                                      a[:, :, 4:8], op)
                    V.tensor_reduce(out_ap, _g3(s4[:], 4), mybir.AxisListType.X, op)

                def gred2(src_ap, op, tag, eng, dt_mid=F32):
                    s4 = g4.tile([128, f // 2], dt_mid, tag=tag + "4")
                    a = _g3(src_ap)
                    eng.tensor_tensor(_g3(s4[:], 4), a[:, :, 0:4],
                                      a[:, :, 4:8], op)
                    s2 = g4.tile([128, f // 4], dt_mid, tag=tag + "2")
                    b = _g3(s4[:], 4)
                    eng.tensor_tensor(_g3(s2[:], 2), b[:, :, 0:2], b[:, :, 2:4], op)
                    s1 = gr.tile([128, R], F32, tag=tag + "1")
                    V.tensor_reduce(s1[:], _g3(s2[:], 2), mybir.AxisListType.X, op)
                    return s1

                gred4(Ef, AL.max, "M", MMn[:, 0:R], V)
                gred4(Ef, AL.min, "N", MMn[:, R:2 * R], V)
                A1 = gred2(Ef, AL.add, "A", P)
                C1 = gred2(podd[:], AL.add, "C", V)
                Ssh = gred2(shalf[:], AL.add, "S", P, dt_mid=BF16)

                # --- small-tile decision chain ---
                # pw2 halves: [C1+MAGIC | C1+MAGIC + (4-Ssh)]
                pw2 = gr.tile([128, 2 * R], F32, tag="pw2")
                P.tensor_scalar(pw2[:, 0:R], C1[:], MAGIC, None, AL.add)
                Nn = gr.tile([128, R], F32, tag="Nn")
                V.tensor_scalar(Nn[:], Ssh[:], -1.0, 4.0, AL.mult, AL.add)
                P.tensor_tensor(pw2[:, R:2 * R], pw2[:, 0:R], Nn[:], AL.add)
                p12 = gr.tile([128, 2 * R], I32, tag="p12")
                V.tensor_scalar(p12[:], pw2[:].bitcast(I32), 1, None, AL.bitwise_and)
                p12f = gr.tile([128, 2 * R], F32, tag="p12f")
                V.tensor_scalar(p12f[:], p12[:], 1.0, None, AL.mult)
                # mm12: [2M-1 | -2Mn]; ch12 = mm12*p12f = [c1 | c2]
                mm12 = gr.tile([128, 2 * R], F32, tag="mm12")
                P.tensor_scalar(mm12[:, 0:R], MMn[:, 0:R], 2.0, -1.0, AL.mult, AL.add)
                V.tensor_scalar(mm12[:, R:2 * R], MMn[:, R:2 * R], -2.0, None, AL.mult)
                ch12 = gr.tile([128, 2 * R], F32, tag="ch12")
                P.tensor_tensor(ch12[:], mm12[:], p12f[:], AL.mult)
                ccd = gr.tile([128, R], F32, tag="ccd")
                P.tensor_tensor(ccd[:], ch12[:, 0:R], ch12[:, R:2 * R], AL.subtract)
                cw = gr.tile([128, R], I32, tag="cw")
                V.scalar_tensor_tensor(cw[:], ccd[:], 2.0, A1[:], AL.add, AL.is_lt)
                # Tsel = cw ? Mn*(2p2-1) : M*(2p1-1)
                i12 = gr.tile([128, 2 * R], F32, tag="i12")
                V.tensor_scalar(i12[:], p12f[:], 2.0, -1.0, AL.mult, AL.add)
                tc12 = gr.tile([128, 2 * R], F32, tag="tc12")
                P.tensor_tensor(tc12[:], MMn[:], i12[:], AL.mult)
                t1c = tc12[:, 0:R]
                V.copy_predicated(t1c, cw[:], tc12[:, R:2 * R])  # Tsel
                # pm2 = 2-4*cw in {2,-2}; cwb = cw in {1,0} (bf16)
                pm2 = gr.tile([128, R], BF16, tag="pm2")
                V.tensor_scalar(pm2[:], cw[:], -4.0, 2.0, AL.mult, AL.add)
                cwb = gr.tile([128, R], BF16, tag="cwb")
                V.tensor_scalar(cwb[:], cw[:], 1.0, None, AL.mult)

                # --- composition: sG = (2*mt*pm + cwb) * shalf = s*sigma ---
                mt = h16.tile([128, f], BF16, tag="mt")
                V.tensor_tensor(_g3(mt[:]), _g3(Ef), _bc(t1c), AL.is_equal)
                sa = h16.tile([128, f], BF16, tag="sa")
                P.tensor_tensor(_g3(sa[:]), _g3(mt[:]), _bc(pm2[:]), AL.mult)
                sg = h16.tile([128, f], BF16, tag="sg")
                P.tensor_tensor(_g3(sg[:]), _g3(sa[:]), _bc(cwb[:]), AL.add)
                sG = h16.tile([128, f], BF16, tag="sG")
                P.tensor_tensor(sG[:], sg[:], shalf[:], AL.mult)

                op_ = ps.tile([128, f], F32, tag="op_")
                for b0 in range(0, f, 512):
                    sl = slice(b0, b0 + 512)
                    nc.tensor.matmul(op_[:, sl], ident[:], f1[:, sl],
                                     start=True, stop=False)
                    nc.tensor.matmul(op_[:, sl], ident[:], sG[:, sl],
                                     start=False, stop=True)
                out = io.tile([128, f], F32, tag="out")
                nc.scalar.activation(out[:], op_[:], AF.Copy)
                nc.sync.dma_start(yt[t], out[:])

    if fix_multiwaits:
        _split_multiwaits(nc)
    return nc


_NC_CACHE = {}


def _get_nc(rows, f):
    key = (rows, f)
    if key not in _NC_CACHE:
        _NC_CACHE[key] = build_nc(rows, f)
    return _NC_CACHE[key]


def kernel(x: np.ndarray, _trace=False) -> np.ndarray:
    assert x.shape == (N_ROWS_FULL, DIM), x.shape
    x = np.ascontiguousarray(np.asarray(x, dtype=np.float32))
    nc = _get_nc(ROWS, F)
    in_maps = [
        {"x": np.ascontiguousarray(x[i * ROWS:(i + 1) * ROWS])}
        for i in range(NCORES)
    ]
    res = run_bass_kernel_spmd(nc, in_maps, core_ids=list(range(NCORES)),
                               trace=_trace)
    out = np.empty_like(x)
    for i in range(NCORES):
        out[i * ROWS:(i + 1) * ROWS] = res.results[i]["y"]
    return out


# revision 4
# speedup vs baseline: 1.0244x; 1.0244x over previous
"""Self-contained E8 lattice quantizer for Trainium2 (8 NeuronCores), v2.

kernel(x) -> nearest-E8-point of each row of x [8388608, 8] f32.

Algorithm (per group of 8 contiguous elements = one row):
  t1 = x + MAGIC ; f1 = t1 - MAGIC          (round-half-even via magic const)
  d1 = x - f1
  E  = (d1.bits & 0x7FFFFFF8) | (7 - idx)   (abs-bits + tie-break index)
  group stats via strided pairwise step + reduce4:
    M = max(E)  -> argmax|d1| encode        (coset-1 nudge target)
    Mn = min(E) -> argmin|d1| encode        (coset-2 nudge target)
    A1s = sum(E as f32) ~= sum|d1|
    Sf1 = sum(f1) bf16-exact, Ssh = sum(+-0.5 sign halves) -> neg count
  small [128,R] chain: parities via MAGIC-LSB, coset decision
    cw <=> par1*(2M-1) + 2 + 2*par2*Mn < A1s   (q2 < q1)
    Tsel = cw ? Mn*(2*par2-1) : M*(2*par1-1)   (negative => never matches)
  mt = (E == bc(Tsel)) as bf16; sG = (2*mt*bc(pm) + bc(cw)) * (+-0.5 by sign)
  out = f1 + sG via PE identity-matmul accumulate into PSUM (ACT evacuates)

Engine split balances ACT (rounding) / Pool / DVE per the v1 cost model.
Sharding: rows split evenly across 8 cores (data parallel, no comms).
"""
import numpy as np
import concourse.bass as bass
import concourse.mybir as mybir
from concourse.tile import TileContext
from concourse.bass_utils import run_bass_kernel_spmd

AL = mybir.AluOpType
AF = mybir.ActivationFunctionType
F32 = mybir.dt.float32
I32 = mybir.dt.int32
U16 = mybir.dt.uint16
BF16 = mybir.dt.bfloat16
MAGIC = float(np.float32(12582912.0))  # 1.5 * 2^23

N_ROWS_FULL = 8388608
DIM = 8
NCORES = 8
ROWS = N_ROWS_FULL // NCORES
F = 1024  # free-dim elems per partition per tile


def _split_multiwaits(nc):
    """This walrus build rejects >1 sem wait per instruction: hoist extras
    onto standalone nops inserted immediately before."""
    n = 0
    for f in nc.m.functions:
        for bb in f.blocks:
            newlist = []
            for ins in bb.instructions:
                si = getattr(ins, "sync_info", None)
                if si is not None and si.on_wait is not None and len(si.on_wait) > 1:
                    waits = list(si.on_wait)
                    for w in waits[:-1]:
                        nop = mybir.InstNoOp(name=f"I-mwfix-{n}", ins=[], outs=[])
                        n += 1
                        nop.engine = ins.engine
                        nop.sync_info = mybir.SyncInfo(on_wait=[w], on_update=[])
                        newlist.append(nop)
                    si.on_wait = [waits[-1]]
                newlist.append(ins)
            bb.instructions = newlist
    return n


def _g3(ap, c=8):
    return ap.rearrange("p (r c) -> p r c", c=c)


def _bc(ap_2d, c=8):
    p, r = ap_2d.shape
    return ap_2d.unsqueeze(2).broadcast_to((p, r, c))


def build_nc(rows=ROWS, f=F, num_devices=NCORES, fix_multiwaits=True):
    elems = rows * DIM
    assert elems % (128 * f) == 0
    ntiles = elems // (128 * f)
    R = f // 8

    nc = bass.Bass("TRN2", num_devices=num_devices, debug=False)
    x = nc.dram_tensor("x", [rows, DIM], F32, kind="ExternalInput")
    y = nc.dram_tensor("y", [rows, DIM], F32, kind="ExternalOutput")
    xt = x[:].flatten().rearrange("(t p f) -> t p f", p=128, f=f)
    yt = y[:].flatten().rearrange("(t p f) -> t p f", p=128, f=f)

    with TileContext(nc) as tc:
        with tc.tile_pool(name="cst", bufs=1) as cst, \
             tc.tile_pool(name="io", bufs=4) as io, \
             tc.tile_pool(name="wk", bufs=4) as wk, \
             tc.tile_pool(name="h16", bufs=4) as h16, \
             tc.tile_pool(name="g4", bufs=2) as g4, \
             tc.tile_pool(name="gr", bufs=3) as gr, \
             tc.tile_pool(name="ps", bufs=2, space="PSUM") as ps:

            # constant: (7 - idx%8) repeating along free dim
            idxf = cst.tile([128, f], I32)
            nc.gpsimd.iota(idxf[:], pattern=[[0, R], [1, 8]], base=0,
                           channel_multiplier=0)
            idxr = cst.tile([128, f], I32)
            nc.vector.tensor_scalar(idxr[:], idxf[:], -1, 7, AL.mult, AL.add)
            ii = cst.tile([128, 128], I32)
            nc.gpsimd.iota(ii[:], pattern=[[0, 128]], base=0, channel_multiplier=1)
            jj = cst.tile([128, 128], I32)
            nc.gpsimd.iota(jj[:], pattern=[[1, 128]], base=0, channel_multiplier=0)
            ident = cst.tile([128, 128], BF16)
            nc.vector.tensor_tensor(ident[:], ii[:], jj[:], AL.is_equal)
            mskA = cst.tile([128, 1], I32)
            nc.vector.memset(mskA[:], 0x7FFFFFF8)

            V, P = nc.vector, nc.gpsimd

            for t in range(ntiles):
                xv = io.tile([128, f], F32, tag="xv")
                nc.sync.dma_start(xv[:], xt[t])

                # rounding (ACT)
                t1 = wk.tile([128, f], F32, tag="t1")
                nc.scalar.activation(t1[:], xv[:], AF.Copy, bias=MAGIC)
                f1 = h16.tile([128, f], BF16, tag="f1")
                nc.scalar.activation(f1[:], t1[:], AF.Copy, bias=-MAGIC)

                # d1 (Pool)
                d1 = wk.tile([128, f], F32, tag="d1")
                P.tensor_tensor(d1[:], xv[:], f1[:], AL.subtract)
                d1i = d1[:].bitcast(I32)

                # E encode (DVE): (d1 & 0x7FFFFFF8) | idxr
                E = wk.tile([128, f], I32, tag="E")
                V.scalar_tensor_tensor(E[:], d1i, mskA[:, 0:1], idxr[:],
                                       AL.bitwise_and, AL.bitwise_or)
                Ef = E[:].bitcast(F32)

                # shalf = +-0.5 by sign of d1 (Pool, arith)
                shalf = h16.tile([128, f], BF16, tag="shalf")
                P.tensor_scalar(shalf[:], d1[:], 0.0, 0.5, AL.is_ge, AL.subtract)

                # --- group reductions ---
                MMn = gr.tile([128, 2 * R], F32, tag="MMn")

                def gred4(src_ap, op, tag, out_ap, eng):
                    s4 = g4.tile([128, f // 2], F32, tag=tag + "4")
                    a = _g3(src_ap)
                    eng.tensor_tensor(_g3(s4[:], 4), a[:, :, 0:4],
                                      a[:, :, 4:8], op)
                    V.tensor_reduce(out_ap, _g3(s4[:], 4), mybir.AxisListType.X, op)

                def gred2(src_ap, op, tag, eng, dt_mid=F32):
                    s4 = g4.tile([128, f // 2], dt_mid, tag=tag + "4")
                    a = _g3(src_ap)
                    eng.tensor_tensor(_g3(s4[:], 4), a[:, :, 0:4],
                                      a[:, :, 4:8], op)
                    s2 = g4.tile([128, f // 4], dt_mid, tag=tag + "2")
                    b = _g3(s4[:], 4)
                    eng.tensor_tensor(_g3(s2[:], 2), b[:, :, 0:2], b[:, :, 2:4], op)
                    s1 = gr.tile([128, R], F32, tag=tag + "1")
                    V.tensor_reduce(s1[:], _g3(s2[:], 2), mybir.AxisListType.X, op)
                    return s1

                gred4(Ef, AL.max, "M", MMn[:, 0:R], V)
                gred4(Ef, AL.min, "N", MMn[:, R:2 * R], V)
                A1 = gred2(Ef, AL.add, "A", P)
                Sf1 = gred2(f1[:], AL.add, "C", P, dt_mid=BF16)
                Ssh = gred2(shalf[:], AL.add, "S", P, dt_mid=BF16)

                # --- small-tile decision chain ---
                # pw2 halves: [C1+MAGIC | C1+MAGIC + (4-Ssh)]
                pw2 = gr.tile([128, 2 * R], F32, tag="pw2")
                P.tensor_scalar(pw2[:, 0:R], Sf1[:], MAGIC, None, AL.add)
                Nn = gr.tile([128, R], F32, tag="Nn")
                P.tensor_scalar(Nn[:], Ssh[:], -1.0, 4.0, AL.mult, AL.add)
                P.tensor_tensor(pw2[:, R:2 * R], pw2[:, 0:R], Nn[:], AL.add)
                p12 = gr.tile([128, 2 * R], I32, tag="p12")
                V.tensor_scalar(p12[:], pw2[:].bitcast(I32), 1, None, AL.bitwise_and)
                p12f = gr.tile([128, 2 * R], F32, tag="p12f")
                V.tensor_scalar(p12f[:], p12[:], 1.0, None, AL.mult)
                # mm12: [2M-1 | -2Mn]; ch12 = mm12*p12f = [c1 | c2]
                mm12 = gr.tile([128, 2 * R], F32, tag="mm12")
                P.tensor_scalar(mm12[:, 0:R], MMn[:, 0:R], 2.0, -1.0, AL.mult, AL.add)
                P.tensor_scalar(mm12[:, R:2 * R], MMn[:, R:2 * R], -2.0, None, AL.mult)
                ch12 = gr.tile([128, 2 * R], F32, tag="ch12")
                P.tensor_tensor(ch12[:], mm12[:], p12f[:], AL.mult)
                ccd = gr.tile([128, R], F32, tag="ccd")
                P.tensor_tensor(ccd[:], ch12[:, 0:R], ch12[:, R:2 * R], AL.subtract)
                cw = gr.tile([128, R], I32, tag="cw")
                V.scalar_tensor_tensor(cw[:], ccd[:], 2.0, A1[:], AL.add, AL.is_lt)
                # Tsel = cw ? Mn*(2p2-1) : M*(2p1-1)
                i12 = gr.tile([128, 2 * R], F32, tag="i12")
                P.tensor_scalar(i12[:], p12f[:], 2.0, -1.0, AL.mult, AL.add)
                tc12 = gr.tile([128, 2 * R], F32, tag="tc12")
                P.tensor_tensor(tc12[:], MMn[:], i12[:], AL.mult)
                t1c = tc12[:, 0:R]
                V.copy_predicated(t1c, cw[:], tc12[:, R:2 * R])  # Tsel
                # pm2 = 2-4*cw in {2,-2}; cwb = cw in {1,0} (bf16)
                pm2 = gr.tile([128, R], BF16, tag="pm2")
                V.tensor_scalar(pm2[:], cw[:], -4.0, 2.0, AL.mult, AL.add)
                cwb = gr.tile([128, R], BF16, tag="cwb")
                V.tensor_scalar(cwb[:], cw[:], 1.0, None, AL.mult)

                # --- composition: sG = (2*mt*pm + cwb) * shalf = s*sigma ---
                mt = h16.tile([128, f], BF16, tag="mt")
                V.tensor_tensor(_g3(mt[:]), _g3(Ef), _bc(t1c), AL.is_equal)
                sa = h16.tile([128, f], BF16, tag="sa")
                P.tensor_tensor(_g3(sa[:]), _g3(mt[:]), _bc(pm2[:]), AL.mult)
                sg = h16.tile([128, f], BF16, tag="sg")
                P.tensor_tensor(_g3(sg[:]), _g3(sa[:]), _bc(cwb[:]), AL.add)
                sG = h16.tile([128, f], BF16, tag="sG")
                V.tensor_tensor(sG[:], sg[:], shalf[:], AL.mult)

                op_ = ps.tile([128, f], F32, tag="op_")
                for b0 in range(0, f, 512):
                    sl = slice(b0, b0 + 512)
                    nc.tensor.matmul(op_[:, sl], ident[:], f1[:, sl],
                                     start=True, stop=False)
                    nc.tensor.matmul(op_[:, sl], ident[:], sG[:, sl],
                                     start=False, stop=True)
                out = io.tile([128, f], F32, tag="out")
                nc.scalar.activation(out[:], op_[:], AF.Copy)
                nc.sync.dma_start(yt[t], out[:])

    if fix_multiwaits:
        _split_multiwaits(nc)
    return nc


_NC_CACHE = {}


def _get_nc(rows, f):
    key = (rows, f)
    if key not in _NC_CACHE:
        _NC_CACHE[key] = build_nc(rows, f)
    return _NC_CACHE[key]


def kernel(x: np.ndarray, _trace=False) -> np.ndarray:
    assert x.shape == (N_ROWS_FULL, DIM), x.shape
    x = np.ascontiguousarray(np.asarray(x, dtype=np.float32))
    nc = _get_nc(ROWS, F)
    in_maps = [
        {"x": np.ascontiguousarray(x[i * ROWS:(i + 1) * ROWS])}
        for i in range(NCORES)
    ]
    res = run_bass_kernel_spmd(nc, in_maps, core_ids=list(range(NCORES)),
                               trace=_trace)
    out = np.empty_like(x)
    for i in range(NCORES):
        out[i * ROWS:(i + 1) * ROWS] = res.results[i]["y"]
    return out


# revision 5
# speedup vs baseline: 1.1253x; 1.0985x over previous
"""Self-contained E8 lattice quantizer for Trainium2 (8 NeuronCores), v2.

kernel(x) -> nearest-E8-point of each row of x [8388608, 8] f32.

Algorithm (per group of 8 contiguous elements = one row):
  t1 = x + MAGIC ; f1 = t1 - MAGIC          (round-half-even via magic const)
  d1 = x - f1
  E  = (d1.bits & 0x7FFFFFF8) | (7 - idx)   (abs-bits + tie-break index)
  group stats via strided pairwise step + reduce4:
    M = max(E)  -> argmax|d1| encode        (coset-1 nudge target)
    Mn = min(E) -> argmin|d1| encode        (coset-2 nudge target)
    A1s = sum(E as f32) ~= sum|d1|
    Sf1 = sum(f1) bf16-exact, Ssh = sum(+-0.5 sign halves) -> neg count
  small [128,R] chain: parities via MAGIC-LSB, coset decision
    cw <=> par1*(2M-1) + 2 + 2*par2*Mn < A1s   (q2 < q1)
    Tsel = cw ? Mn*(2*par2-1) : M*(2*par1-1)   (negative => never matches)
  mt = (E == bc(Tsel)) as bf16; sG = (2*mt*bc(pm) + bc(cw)) * (+-0.5 by sign)
  out = f1 + sG via PE identity-matmul accumulate into PSUM (ACT evacuates)

Engine split balances ACT (rounding) / Pool / DVE per the v1 cost model.
Sharding: rows split evenly across 8 cores (data parallel, no comms).
"""
import numpy as np
import concourse.bass as bass
import concourse.mybir as mybir
from concourse.tile import TileContext
from concourse.bass_utils import run_bass_kernel_spmd

AL = mybir.AluOpType
AF = mybir.ActivationFunctionType
F32 = mybir.dt.float32
I32 = mybir.dt.int32
U16 = mybir.dt.uint16
BF16 = mybir.dt.bfloat16
MAGIC = float(np.float32(12582912.0))  # 1.5 * 2^23

N_ROWS_FULL = 8388608
DIM = 8
NCORES = 8
ROWS = N_ROWS_FULL // NCORES
F = 1024  # free-dim elems per partition per tile


def _split_multiwaits(nc):
    """This walrus build rejects >1 sem wait per instruction: hoist extras
    onto standalone nops inserted immediately before."""
    n = 0
    for f in nc.m.functions:
        for bb in f.blocks:
            newlist = []
            for ins in bb.instructions:
                si = getattr(ins, "sync_info", None)
                if si is not None and si.on_wait is not None and len(si.on_wait) > 1:
                    waits = list(si.on_wait)
                    for w in waits[:-1]:
                        nop = mybir.InstNoOp(name=f"I-mwfix-{n}", ins=[], outs=[])
                        n += 1
                        nop.engine = ins.engine
                        nop.sync_info = mybir.SyncInfo(on_wait=[w], on_update=[])
                        newlist.append(nop)
                    si.on_wait = [waits[-1]]
                newlist.append(ins)
            bb.instructions = newlist
    return n


def _g3(ap, c=8):
    return ap.rearrange("p (r c) -> p r c", c=c)


def _bc(ap_2d, c=8):
    p, r = ap_2d.shape
    return ap_2d.unsqueeze(2).broadcast_to((p, r, c))


def build_nc(rows=ROWS, f=F, num_devices=NCORES, fix_multiwaits=True):
    elems = rows * DIM
    assert elems % (128 * f) == 0
    ntiles = elems // (128 * f)
    R = f // 8

    nc = bass.Bass("TRN2", num_devices=num_devices, debug=False)
    x = nc.dram_tensor("x", [rows, DIM], F32, kind="ExternalInput")
    y = nc.dram_tensor("y", [rows, DIM], F32, kind="ExternalOutput")
    xt = x[:].flatten().rearrange("(t p f) -> t p f", p=128, f=f)
    yt = y[:].flatten().rearrange("(t p f) -> t p f", p=128, f=f)

    with TileContext(nc) as tc:
        with tc.tile_pool(name="cst", bufs=1) as cst, \
             tc.tile_pool(name="io", bufs=6) as io, \
             tc.tile_pool(name="wk", bufs=4) as wk, \
             tc.tile_pool(name="h16", bufs=4) as h16, \
             tc.tile_pool(name="g4", bufs=2) as g4, \
             tc.tile_pool(name="gr", bufs=3) as gr, \
             tc.tile_pool(name="ps", bufs=2, space="PSUM") as ps:

            # constant: (7 - idx%8) repeating along free dim
            idxf = cst.tile([128, f], I32)
            nc.gpsimd.iota(idxf[:], pattern=[[0, R], [1, 8]], base=0,
                           channel_multiplier=0)
            idxr = cst.tile([128, f], I32)
            nc.vector.tensor_scalar(idxr[:], idxf[:], -1, 7, AL.mult, AL.add)
            ii = cst.tile([128, 128], I32)
            nc.gpsimd.iota(ii[:], pattern=[[0, 128]], base=0, channel_multiplier=1)
            jj = cst.tile([128, 128], I32)
            nc.gpsimd.iota(jj[:], pattern=[[1, 128]], base=0, channel_multiplier=0)
            ident = cst.tile([128, 128], BF16)
            nc.vector.tensor_tensor(ident[:], ii[:], jj[:], AL.is_equal)
            mskA = cst.tile([128, 1], I32)
            nc.vector.memset(mskA[:], 0x7FFFFFF8)

            V, P = nc.vector, nc.gpsimd

            for t in range(ntiles):
                xv = io.tile([128, f], F32, tag="xv")
                nc.sync.dma_start(xv[:], xt[t])

                # rounding (ACT)
                t1 = wk.tile([128, f], F32, tag="t1")
                nc.scalar.activation(t1[:], xv[:], AF.Copy, bias=MAGIC)
                f1 = h16.tile([128, f], BF16, tag="f1")
                nc.scalar.activation(f1[:], t1[:], AF.Copy, bias=-MAGIC)

                # d1 (Pool)
                d1 = wk.tile([128, f], F32, tag="d1")
                P.tensor_tensor(d1[:], xv[:], f1[:], AL.subtract)
                d1i = d1[:].bitcast(I32)

                # E encode (DVE): (d1 & 0x7FFFFFF8) | idxr
                E = wk.tile([128, f], I32, tag="E")
                V.scalar_tensor_tensor(E[:], d1i, mskA[:, 0:1], idxr[:],
                                       AL.bitwise_and, AL.bitwise_or)
                Ef = E[:].bitcast(F32)

                # shalf = +-0.5 by sign of d1 (Pool, arith)
                shalf = h16.tile([128, f], BF16, tag="shalf")
                P.tensor_scalar(shalf[:], d1[:], 0.0, 0.5, AL.is_ge, AL.subtract)

                # --- group reductions ---
                MMn = gr.tile([128, 2 * R], F32, tag="MMn")

                def gred4(src_ap, op, tag, out_ap, eng):
                    s4 = g4.tile([128, f // 2], F32, tag=tag + "4")
                    a = _g3(src_ap)
                    eng.tensor_tensor(_g3(s4[:], 4), a[:, :, 0:4],
                                      a[:, :, 4:8], op)
                    V.tensor_reduce(out_ap, _g3(s4[:], 4), mybir.AxisListType.X, op)

                def gred2(src_ap, op, tag, eng, dt_mid=F32):
                    s4 = g4.tile([128, f // 2], dt_mid, tag=tag + "4")
                    a = _g3(src_ap)
                    eng.tensor_tensor(_g3(s4[:], 4), a[:, :, 0:4],
                                      a[:, :, 4:8], op)
                    s2 = g4.tile([128, f // 4], dt_mid, tag=tag + "2")
                    b = _g3(s4[:], 4)
                    eng.tensor_tensor(_g3(s2[:], 2), b[:, :, 0:2], b[:, :, 2:4], op)
                    s1 = gr.tile([128, R], F32, tag=tag + "1")
                    V.tensor_reduce(s1[:], _g3(s2[:], 2), mybir.AxisListType.X, op)
                    return s1

                gred4(Ef, AL.max, "M", MMn[:, 0:R], V)
                gred4(Ef, AL.min, "N", MMn[:, R:2 * R], V)
                A1 = gred2(Ef, AL.add, "A", P)
                Sf1 = gred2(f1[:], AL.add, "C", P, dt_mid=BF16)
                Ssh = gred2(shalf[:], AL.add, "S", P, dt_mid=BF16)

                # --- small-tile decision chain ---
                # pw2 halves: [C1+MAGIC | C1+MAGIC + (4-Ssh)]
                pw2 = gr.tile([128, 2 * R], F32, tag="pw2")
                P.tensor_scalar(pw2[:, 0:R], Sf1[:], MAGIC, None, AL.add)
                Nn = gr.tile([128, R], F32, tag="Nn")
                P.tensor_scalar(Nn[:], Ssh[:], -1.0, 4.0, AL.mult, AL.add)
                P.tensor_tensor(pw2[:, R:2 * R], pw2[:, 0:R], Nn[:], AL.add)
                p12 = gr.tile([128, 2 * R], I32, tag="p12")
                V.tensor_scalar(p12[:], pw2[:].bitcast(I32), 1, None, AL.bitwise_and)
                p12f = gr.tile([128, 2 * R], F32, tag="p12f")
                V.tensor_scalar(p12f[:], p12[:], 1.0, None, AL.mult)
                # mm12: [2M-1 | -2Mn]; ch12 = mm12*p12f = [c1 | c2]
                mm12 = gr.tile([128, 2 * R], F32, tag="mm12")
                P.tensor_scalar(mm12[:, 0:R], MMn[:, 0:R], 2.0, -1.0, AL.mult, AL.add)
                P.tensor_scalar(mm12[:, R:2 * R], MMn[:, R:2 * R], -2.0, None, AL.mult)
                ch12 = gr.tile([128, 2 * R], F32, tag="ch12")
                P.tensor_tensor(ch12[:], mm12[:], p12f[:], AL.mult)
                ccd = gr.tile([128, R], F32, tag="ccd")
                P.tensor_tensor(ccd[:], ch12[:, 0:R], ch12[:, R:2 * R], AL.subtract)
                cw = gr.tile([128, R], I32, tag="cw")
                V.scalar_tensor_tensor(cw[:], ccd[:], 2.0, A1[:], AL.add, AL.is_lt)
                # Tsel = cw ? Mn*(2p2-1) : M*(2p1-1)
                i12 = gr.tile([128, 2 * R], F32, tag="i12")
                P.tensor_scalar(i12[:], p12f[:], 2.0, -1.0, AL.mult, AL.add)
                tc12 = gr.tile([128, 2 * R], F32, tag="tc12")
                P.tensor_tensor(tc12[:], MMn[:], i12[:], AL.mult)
                t1c = tc12[:, 0:R]
                V.copy_predicated(t1c, cw[:], tc12[:, R:2 * R])  # Tsel
                # pm2 = 2-4*cw in {2,-2}; cwb = cw in {1,0} (bf16)
                pm2 = gr.tile([128, R], BF16, tag="pm2")
                V.tensor_scalar(pm2[:], cw[:], -4.0, 2.0, AL.mult, AL.add)
                cwb = gr.tile([128, R], BF16, tag="cwb")
                V.tensor_scalar(cwb[:], cw[:], 1.0, None, AL.mult)

                # --- composition: sG = (2*mt*pm + cwb) * shalf = s*sigma ---
                mt = h16.tile([128, f], BF16, tag="mt")
                V.tensor_tensor(_g3(mt[:]), _g3(Ef), _bc(t1c), AL.is_equal)
                sa = h16.tile([128, f], BF16, tag="sa")
                P.tensor_tensor(_g3(sa[:]), _g3(mt[:]), _bc(pm2[:]), AL.mult)
                sg = h16.tile([128, f], BF16, tag="sg")
                P.tensor_tensor(_g3(sg[:]), _g3(sa[:]), _bc(cwb[:]), AL.add)
                sG = h16.tile([128, f], BF16, tag="sG")
                V.tensor_tensor(sG[:], sg[:], shalf[:], AL.mult)

                op_ = ps.tile([128, f], F32, tag="op_")
                for b0 in range(0, f, 512):
                    sl = slice(b0, b0 + 512)
                    nc.tensor.matmul(op_[:, sl], ident[:], f1[:, sl],
                                     start=True, stop=False)
                    nc.tensor.matmul(op_[:, sl], ident[:], sG[:, sl],
                                     start=False, stop=True)
                out = io.tile([128, f], F32, tag="out")
                nc.scalar.activation(out[:], op_[:], AF.Copy)
                nc.sync.dma_start(yt[t], out[:])

    if fix_multiwaits:
        _split_multiwaits(nc)
    return nc


_NC_CACHE = {}


def _get_nc(rows, f):
    key = (rows, f)
    if key not in _NC_CACHE:
        _NC_CACHE[key] = build_nc(rows, f)
    return _NC_CACHE[key]


def kernel(x: np.ndarray, _trace=False) -> np.ndarray:
    assert x.shape == (N_ROWS_FULL, DIM), x.shape
    x = np.ascontiguousarray(np.asarray(x, dtype=np.float32))
    nc = _get_nc(ROWS, F)
    in_maps = [
        {"x": np.ascontiguousarray(x[i * ROWS:(i + 1) * ROWS])}
        for i in range(NCORES)
    ]
    res = run_bass_kernel_spmd(nc, in_maps, core_ids=list(range(NCORES)),
                               trace=_trace)
    out = np.empty_like(x)
    for i in range(NCORES):
        out[i * ROWS:(i + 1) * ROWS] = res.results[i]["y"]
    return out


# revision 7
# speedup vs baseline: 1.1711x; 1.0407x over previous
"""Self-contained E8 lattice quantizer for Trainium2 (8 NeuronCores), v2.

kernel(x) -> nearest-E8-point of each row of x [8388608, 8] f32.

Algorithm (per group of 8 contiguous elements = one row):
  t1 = x + MAGIC ; f1 = t1 - MAGIC          (round-half-even via magic const)
  d1 = x - f1
  E  = (d1.bits & 0x7FFFFFF8) | (7 - idx)   (abs-bits + tie-break index)
  group stats via strided pairwise step + reduce4:
    M = max(E)  -> argmax|d1| encode        (coset-1 nudge target)
    Mn = min(E) -> argmin|d1| encode        (coset-2 nudge target)
    A1s = sum(E as f32) ~= sum|d1|
    Sf1 = sum(f1) bf16-exact, Ssh = sum(+-0.5 sign halves) -> neg count
  small [128,R] chain: parities via MAGIC-LSB, coset decision
    cw <=> par1*(2M-1) + 2 + 2*par2*Mn < A1s   (q2 < q1)
    Tsel = cw ? Mn*(2*par2-1) : M*(2*par1-1)   (negative => never matches)
  mt = (E == bc(Tsel)) as bf16; sG = (2*mt*bc(pm) + bc(cw)) * (+-0.5 by sign)
  out = f1 + sG via PE identity-matmul accumulate into PSUM (ACT evacuates)

Engine split balances ACT (rounding) / Pool / DVE per the v1 cost model.
Sharding: rows split evenly across 8 cores (data parallel, no comms).
"""
import numpy as np
import concourse.bass as bass
import concourse.mybir as mybir
from concourse.tile import TileContext
from concourse.bass_utils import run_bass_kernel_spmd

AL = mybir.AluOpType
AF = mybir.ActivationFunctionType
F32 = mybir.dt.float32
I32 = mybir.dt.int32
U16 = mybir.dt.uint16
BF16 = mybir.dt.bfloat16
MAGIC = float(np.float32(12582912.0))  # 1.5 * 2^23

N_ROWS_FULL = 8388608
DIM = 8
NCORES = 8
ROWS = N_ROWS_FULL // NCORES
F = 1024  # free-dim elems per partition per tile


def _split_multiwaits(nc):
    """This walrus build rejects >1 sem wait per instruction: hoist extras
    onto standalone nops inserted immediately before."""
    n = 0
    for f in nc.m.functions:
        for bb in f.blocks:
            newlist = []
            for ins in bb.instructions:
                si = getattr(ins, "sync_info", None)
                if si is not None and si.on_wait is not None and len(si.on_wait) > 1:
                    waits = list(si.on_wait)
                    for w in waits[:-1]:
                        nop = mybir.InstNoOp(name=f"I-mwfix-{n}", ins=[], outs=[])
                        n += 1
                        nop.engine = ins.engine
                        nop.sync_info = mybir.SyncInfo(on_wait=[w], on_update=[])
                        newlist.append(nop)
                    si.on_wait = [waits[-1]]
                newlist.append(ins)
            bb.instructions = newlist
    return n


def _g3(ap, c=8):
    return ap.rearrange("p (r c) -> p r c", c=c)


def _bc(ap_2d, c=8):
    p, r = ap_2d.shape
    return ap_2d.unsqueeze(2).broadcast_to((p, r, c))


def build_nc(rows=ROWS, f=F, num_devices=NCORES, fix_multiwaits=True):
    elems = rows * DIM
    assert elems % (128 * f) == 0
    ntiles = elems // (128 * f)
    R = f // 8

    nc = bass.Bass("TRN2", num_devices=num_devices, debug=False)
    x = nc.dram_tensor("x", [rows, DIM], F32, kind="ExternalInput")
    y = nc.dram_tensor("y", [rows, DIM], F32, kind="ExternalOutput")
    xt = x[:].flatten().rearrange("(t p f) -> t p f", p=128, f=f)
    yt = y[:].flatten().rearrange("(t p f) -> t p f", p=128, f=f)

    with TileContext(nc) as tc:
        with tc.tile_pool(name="cst", bufs=1) as cst, \
             tc.tile_pool(name="io", bufs=5) as io, \
             tc.tile_pool(name="wk", bufs=4) as wk, \
             tc.tile_pool(name="h16", bufs=4) as h16, \
             tc.tile_pool(name="g4", bufs=2) as g4, \
             tc.tile_pool(name="gr", bufs=3) as gr, \
             tc.tile_pool(name="ps", bufs=2, space="PSUM") as ps:

            # constant: (7 - idx%8) repeating along free dim
            idxf = cst.tile([128, f], I32)
            nc.gpsimd.iota(idxf[:], pattern=[[0, R], [1, 8]], base=0,
                           channel_multiplier=0)
            idxr = cst.tile([128, f], I32)
            nc.vector.tensor_scalar(idxr[:], idxf[:], -1, 7, AL.mult, AL.add)
            ii = cst.tile([128, 128], I32)
            nc.gpsimd.iota(ii[:], pattern=[[0, 128]], base=0, channel_multiplier=1)
            jj = cst.tile([128, 128], I32)
            nc.gpsimd.iota(jj[:], pattern=[[1, 128]], base=0, channel_multiplier=0)
            ident = cst.tile([128, 128], BF16)
            nc.vector.tensor_tensor(ident[:], ii[:], jj[:], AL.is_equal)
            mskA = cst.tile([128, 1], I32)
            nc.vector.memset(mskA[:], 0x7FFFFFF8)

            V, P = nc.vector, nc.gpsimd

            for t in range(ntiles):
                xv = io.tile([128, f], F32, tag="xv")
                nc.sync.dma_start(xv[:], xt[t])

                # rounding (ACT)
                t1 = wk.tile([128, f], F32, tag="t1")
                nc.scalar.activation(t1[:], xv[:], AF.Copy, bias=MAGIC)
                f1 = h16.tile([128, f], BF16, tag="f1")
                nc.scalar.activation(f1[:], t1[:], AF.Copy, bias=-MAGIC)

                # d1 (Pool)
                d1 = wk.tile([128, f], F32, tag="d1")
                P.tensor_tensor(d1[:], xv[:], f1[:], AL.subtract)
                d1i = d1[:].bitcast(I32)

                # E encode (DVE): (d1 & 0x7FFFFFF8) | idxr
                E = wk.tile([128, f], I32, tag="E")
                V.scalar_tensor_tensor(E[:], d1i, mskA[:, 0:1], idxr[:],
                                       AL.bitwise_and, AL.bitwise_or)
                Ef = E[:].bitcast(F32)

                # shalf = +-0.5 by sign of d1 (Pool, arith)
                shalf = h16.tile([128, f], BF16, tag="shalf")
                P.tensor_scalar(shalf[:], d1[:], 0.0, 0.5, AL.is_ge, AL.subtract)

                # --- group reductions ---
                MMn = gr.tile([128, 2 * R], F32, tag="MMn")

                def gtree(src_ap, op, tag, out_ap, e1, e2, e3, dt_mid=F32):
                    s4 = g4.tile([128, f // 2], dt_mid, tag=tag + "4")
                    a = _g3(src_ap)
                    e1.tensor_tensor(_g3(s4[:], 4), a[:, :, 0:4],
                                     a[:, :, 4:8], op)
                    s2 = g4.tile([128, f // 4], dt_mid, tag=tag + "2")
                    b = _g3(s4[:], 4)
                    e2.tensor_tensor(_g3(s2[:], 2), b[:, :, 0:2], b[:, :, 2:4], op)
                    c = _g3(s2[:], 2)
                    e3.tensor_tensor(out_ap.unsqueeze(2), c[:, :, 0:1],
                                     c[:, :, 1:2], op)

                gtree(Ef, AL.max, "M", MMn[:, 0:R], V, V, V)
                gtree(Ef, AL.min, "N", MMn[:, R:2 * R], V, V, V)
                A1 = gr.tile([128, R], F32, tag="A1")
                gtree(Ef, AL.add, "A", A1[:], P, P, P)
                Sf1 = gr.tile([128, R], F32, tag="C1")
                gtree(f1[:], AL.add, "C", Sf1[:], P, P, P, dt_mid=BF16)
                Ssh = gr.tile([128, R], F32, tag="S1")
                gtree(shalf[:], AL.add, "S", Ssh[:], P, P, P, dt_mid=BF16)

                # --- small-tile decision chain ---
                # pw2 halves: [C1+MAGIC | C1+MAGIC + (4-Ssh)]
                pw2 = gr.tile([128, 2 * R], F32, tag="pw2")
                nc.scalar.activation(pw2[:, 0:R], Sf1[:], AF.Copy, bias=MAGIC)
                Nn = gr.tile([128, R], F32, tag="Nn")
                nc.scalar.activation(Nn[:], Ssh[:], AF.Copy, scale=-1.0, bias=4.0)
                P.tensor_tensor(pw2[:, R:2 * R], pw2[:, 0:R], Nn[:], AL.add)
                p12 = gr.tile([128, 2 * R], I32, tag="p12")
                V.tensor_scalar(p12[:], pw2[:].bitcast(I32), 1, None, AL.bitwise_and)
                p12f = gr.tile([128, 2 * R], F32, tag="p12f")
                nc.scalar.activation(p12f[:], p12[:], AF.Copy)
                # mm12: [2M-1 | -2Mn]; ch12 = mm12*p12f = [c1 | c2]
                mm12 = gr.tile([128, 2 * R], F32, tag="mm12")
                nc.scalar.activation(mm12[:, 0:R], MMn[:, 0:R], AF.Copy, scale=2.0, bias=-1.0)
                nc.scalar.activation(mm12[:, R:2 * R], MMn[:, R:2 * R], AF.Copy, scale=-2.0)
                ch12 = gr.tile([128, 2 * R], F32, tag="ch12")
                P.tensor_tensor(ch12[:], mm12[:], p12f[:], AL.mult)
                ccd = gr.tile([128, R], F32, tag="ccd")
                P.tensor_tensor(ccd[:], ch12[:, 0:R], ch12[:, R:2 * R], AL.subtract)
                cw = gr.tile([128, R], I32, tag="cw")
                V.scalar_tensor_tensor(cw[:], ccd[:], 2.0, A1[:], AL.add, AL.is_lt)
                # Tsel = cw ? Mn*(2p2-1) : M*(2p1-1)
                i12 = gr.tile([128, 2 * R], F32, tag="i12")
                nc.scalar.activation(i12[:], p12f[:], AF.Copy, scale=2.0, bias=-1.0)
                tc12 = gr.tile([128, 2 * R], F32, tag="tc12")
                P.tensor_tensor(tc12[:], MMn[:], i12[:], AL.mult)
                t1c = tc12[:, 0:R]
                V.copy_predicated(t1c, cw[:], tc12[:, R:2 * R])  # Tsel
                # pm2 = 2-4*cw in {2,-2}; cwb = cw in {1,0} (bf16)
                pm2 = gr.tile([128, R], BF16, tag="pm2")
                nc.scalar.activation(pm2[:], cw[:], AF.Copy, scale=-4.0, bias=2.0)
                cwb = gr.tile([128, R], BF16, tag="cwb")
                nc.scalar.activation(cwb[:], cw[:], AF.Copy)

                # --- composition: sG = (2*mt*pm + cwb) * shalf = s*sigma ---
                mt = h16.tile([128, f], BF16, tag="mt")
                V.tensor_tensor(_g3(mt[:]), _g3(Ef), _bc(t1c), AL.is_equal)
                sa = h16.tile([128, f], BF16, tag="sa")
                h = f // 2
                P.tensor_tensor(_g3(sa[:, 0:h]), _g3(mt[:, 0:h]),
                                _bc(pm2[:, 0:R // 2]), AL.mult)
                V.tensor_tensor(_g3(sa[:, h:f]), _g3(mt[:, h:f]),
                                _bc(pm2[:, R // 2:R]), AL.mult)
                sg = h16.tile([128, f], BF16, tag="sg")
                q = 3 * f // 4
                P.tensor_tensor(_g3(sg[:, 0:q]), _g3(sa[:, 0:q]),
                                _bc(cwb[:, 0:3 * R // 4]), AL.add)
                V.tensor_tensor(_g3(sg[:, q:f]), _g3(sa[:, q:f]),
                                _bc(cwb[:, 3 * R // 4:R]), AL.add)
                sG = h16.tile([128, f], BF16, tag="sG")
                P.tensor_tensor(sG[:], sg[:], shalf[:], AL.mult)

                op_ = ps.tile([128, f], F32, tag="op_")
                for b0 in range(0, f, 512):
                    sl = slice(b0, b0 + 512)
                    nc.tensor.matmul(op_[:, sl], ident[:], f1[:, sl],
                                     start=True, stop=False)
                    nc.tensor.matmul(op_[:, sl], ident[:], sG[:, sl],
                                     start=False, stop=True)
                out = io.tile([128, f], F32, tag="out")
                nc.scalar.activation(out[:], op_[:], AF.Copy)
                nc.sync.dma_start(yt[t], out[:])

    if fix_multiwaits:
        _split_multiwaits(nc)
    return nc


_NC_CACHE = {}


def _get_nc(rows, f):
    key = (rows, f)
    if key not in _NC_CACHE:
        _NC_CACHE[key] = build_nc(rows, f)
    return _NC_CACHE[key]


def kernel(x: np.ndarray, _trace=False) -> np.ndarray:
    assert x.shape == (N_ROWS_FULL, DIM), x.shape
    x = np.ascontiguousarray(np.asarray(x, dtype=np.float32))
    nc = _get_nc(ROWS, F)
    in_maps = [
        {"x": np.ascontiguousarray(x[i * ROWS:(i + 1) * ROWS])}
        for i in range(NCORES)
    ]
    res = run_bass_kernel_spmd(nc, in_maps, core_ids=list(range(NCORES)),
                               trace=_trace)
    out = np.empty_like(x)
    for i in range(NCORES):
        out[i * ROWS:(i + 1) * ROWS] = res.results[i]["y"]
    return out


# revision 8
# speedup vs baseline: 1.1788x; 1.0065x over previous
"""Self-contained E8 lattice quantizer for Trainium2 (8 NeuronCores), v2.

kernel(x) -> nearest-E8-point of each row of x [8388608, 8] f32.

Algorithm (per group of 8 contiguous elements = one row):
  t1 = x + MAGIC ; f1 = t1 - MAGIC          (round-half-even via magic const)
  d1 = x - f1
  E  = (d1.bits & 0x7FFFFFF8) | (7 - idx)   (abs-bits + tie-break index)
  group stats via strided pairwise step + reduce4:
    M = max(E)  -> argmax|d1| encode        (coset-1 nudge target)
    Mn = min(E) -> argmin|d1| encode        (coset-2 nudge target)
    A1s = sum(E as f32) ~= sum|d1|
    Sf1 = sum(f1) bf16-exact, Ssh = sum(+-0.5 sign halves) -> neg count
  small [128,R] chain: parities via MAGIC-LSB, coset decision
    cw <=> par1*(2M-1) + 2 + 2*par2*Mn < A1s   (q2 < q1)
    Tsel = cw ? Mn*(2*par2-1) : M*(2*par1-1)   (negative => never matches)
  mt = (E == bc(Tsel)) as bf16; sG = (2*mt*bc(pm) + bc(cw)) * (+-0.5 by sign)
  out = f1 + sG via PE identity-matmul accumulate into PSUM (ACT evacuates)

Engine split balances ACT (rounding) / Pool / DVE per the v1 cost model.
Sharding: rows split evenly across 8 cores (data parallel, no comms).
"""
import numpy as np
import concourse.bass as bass
import concourse.mybir as mybir
from concourse.tile import TileContext
from concourse.bass_utils import run_bass_kernel_spmd

AL = mybir.AluOpType
AF = mybir.ActivationFunctionType
F32 = mybir.dt.float32
I32 = mybir.dt.int32
U16 = mybir.dt.uint16
BF16 = mybir.dt.bfloat16
MAGIC = float(np.float32(12582912.0))  # 1.5 * 2^23

N_ROWS_FULL = 8388608
DIM = 8
NCORES = 8
ROWS = N_ROWS_FULL // NCORES
F = 1024  # free-dim elems per partition per tile


def _split_multiwaits(nc):
    """This walrus build rejects >1 sem wait per instruction: hoist extras
    onto standalone nops inserted immediately before."""
    n = 0
    for f in nc.m.functions:
        for bb in f.blocks:
            newlist = []
            for ins in bb.instructions:
                si = getattr(ins, "sync_info", None)
                if si is not None and si.on_wait is not None and len(si.on_wait) > 1:
                    waits = list(si.on_wait)
                    for w in waits[:-1]:
                        nop = mybir.InstNoOp(name=f"I-mwfix-{n}", ins=[], outs=[])
                        n += 1
                        nop.engine = ins.engine
                        nop.sync_info = mybir.SyncInfo(on_wait=[w], on_update=[])
                        newlist.append(nop)
                    si.on_wait = [waits[-1]]
                newlist.append(ins)
            bb.instructions = newlist
    return n


def _g3(ap, c=8):
    return ap.rearrange("p (r c) -> p r c", c=c)


def _bc(ap_2d, c=8):
    p, r = ap_2d.shape
    return ap_2d.unsqueeze(2).broadcast_to((p, r, c))


def build_nc(rows=ROWS, f=F, num_devices=NCORES, fix_multiwaits=True):
    elems = rows * DIM
    assert elems % (128 * f) == 0
    ntiles = elems // (128 * f)
    R = f // 8

    nc = bass.Bass("TRN2", num_devices=num_devices, debug=False)
    x = nc.dram_tensor("x", [rows, DIM], F32, kind="ExternalInput")
    y = nc.dram_tensor("y", [rows, DIM], F32, kind="ExternalOutput")
    xt = x[:].flatten().rearrange("(t p f) -> t p f", p=128, f=f)
    yt = y[:].flatten().rearrange("(t p f) -> t p f", p=128, f=f)

    with TileContext(nc) as tc:
        with tc.tile_pool(name="cst", bufs=1) as cst, \
             tc.tile_pool(name="io", bufs=5) as io, \
             tc.tile_pool(name="wk", bufs=4) as wk, \
             tc.tile_pool(name="h16", bufs=4) as h16, \
             tc.tile_pool(name="g4", bufs=2) as g4, \
             tc.tile_pool(name="gr", bufs=3) as gr, \
             tc.tile_pool(name="ps", bufs=2, space="PSUM") as ps:

            # constant: (7 - idx%8) repeating along free dim
            idxf = cst.tile([128, f], I32)
            nc.gpsimd.iota(idxf[:], pattern=[[0, R], [1, 8]], base=0,
                           channel_multiplier=0)
            idxr = cst.tile([128, f], I32)
            nc.vector.tensor_scalar(idxr[:], idxf[:], -1, 7, AL.mult, AL.add)
            ii = cst.tile([128, 128], I32)
            nc.gpsimd.iota(ii[:], pattern=[[0, 128]], base=0, channel_multiplier=1)
            jj = cst.tile([128, 128], I32)
            nc.gpsimd.iota(jj[:], pattern=[[1, 128]], base=0, channel_multiplier=0)
            ident = cst.tile([128, 128], BF16)
            nc.vector.tensor_tensor(ident[:], ii[:], jj[:], AL.is_equal)
            mskA = cst.tile([128, 1], I32)
            nc.vector.memset(mskA[:], 0x7FFFFFF8)

            V, P = nc.vector, nc.gpsimd

            for t in range(ntiles):
                xv = io.tile([128, f], F32, tag="xv")
                nc.sync.dma_start(xv[:], xt[t])

                # rounding (ACT)
                t1 = wk.tile([128, f], F32, tag="t1")
                nc.scalar.activation(t1[:], xv[:], AF.Copy, bias=MAGIC)
                f1 = h16.tile([128, f], BF16, tag="f1")
                nc.scalar.activation(f1[:], t1[:], AF.Copy, bias=-MAGIC)

                # d1 (Pool)
                d1 = wk.tile([128, f], F32, tag="d1")
                P.tensor_tensor(d1[:], xv[:], f1[:], AL.subtract)
                d1i = d1[:].bitcast(I32)

                # E encode (DVE): (d1 & 0x7FFFFFF8) | idxr
                E = wk.tile([128, f], I32, tag="E")
                V.scalar_tensor_tensor(E[:], d1i, mskA[:, 0:1], idxr[:],
                                       AL.bitwise_and, AL.bitwise_or)
                Ef = E[:].bitcast(F32)

                # shalf = +-0.5 by sign of d1 (Pool, arith)
                shalf = h16.tile([128, f], BF16, tag="shalf")
                P.tensor_scalar(shalf[:], d1[:], 0.0, 0.5, AL.is_ge, AL.subtract)

                # --- group reductions ---
                MMn = gr.tile([128, 2 * R], F32, tag="MMn")

                def gtree(src_ap, op, tag, out_ap, e1, e2, e3, dt_mid=F32):
                    s4 = g4.tile([128, f // 2], dt_mid, tag=tag + "4")
                    a = _g3(src_ap)
                    e1.tensor_tensor(_g3(s4[:], 4), a[:, :, 0:4],
                                     a[:, :, 4:8], op)
                    s2 = g4.tile([128, f // 4], dt_mid, tag=tag + "2")
                    b = _g3(s4[:], 4)
                    e2.tensor_tensor(_g3(s2[:], 2), b[:, :, 0:2], b[:, :, 2:4], op)
                    c = _g3(s2[:], 2)
                    e3.tensor_tensor(out_ap.unsqueeze(2), c[:, :, 0:1],
                                     c[:, :, 1:2], op)

                gtree(Ef, AL.max, "M", MMn[:, 0:R], V, V, V)
                gtree(Ef, AL.min, "N", MMn[:, R:2 * R], V, V, V)
                A1 = gr.tile([128, R], F32, tag="A1")
                gtree(Ef, AL.add, "A", A1[:], P, P, P)
                Sf1 = gr.tile([128, R], F32, tag="C1")
                gtree(f1[:], AL.add, "C", Sf1[:], P, P, P, dt_mid=BF16)
                Ssh = gr.tile([128, R], F32, tag="S1")
                gtree(shalf[:], AL.add, "S", Ssh[:], P, P, P, dt_mid=BF16)

                # --- small-tile decision chain ---
                # pw2 halves: [C1+MAGIC | C1+MAGIC + (4-Ssh)]
                pw2 = gr.tile([128, 2 * R], F32, tag="pw2")
                nc.scalar.activation(pw2[:, 0:R], Sf1[:], AF.Copy, bias=MAGIC)
                Nn = gr.tile([128, R], F32, tag="Nn")
                nc.scalar.activation(Nn[:], Ssh[:], AF.Copy, scale=-1.0, bias=4.0)
                P.tensor_tensor(pw2[:, R:2 * R], pw2[:, 0:R], Nn[:], AL.add)
                p12 = gr.tile([128, 2 * R], I32, tag="p12")
                V.tensor_scalar(p12[:], pw2[:].bitcast(I32), 1, None, AL.bitwise_and)
                p12f = gr.tile([128, 2 * R], F32, tag="p12f")
                nc.scalar.activation(p12f[:], p12[:], AF.Copy)
                # mm12: [2M-1 | -2Mn]; ch12 = mm12*p12f = [c1 | c2]
                mm12 = gr.tile([128, 2 * R], F32, tag="mm12")
                nc.scalar.activation(mm12[:, 0:R], MMn[:, 0:R], AF.Copy, scale=2.0, bias=-1.0)
                nc.scalar.activation(mm12[:, R:2 * R], MMn[:, R:2 * R], AF.Copy, scale=-2.0)
                ch12 = gr.tile([128, 2 * R], F32, tag="ch12")
                P.tensor_tensor(ch12[:], mm12[:], p12f[:], AL.mult)
                ccd = gr.tile([128, R], F32, tag="ccd")
                P.tensor_tensor(ccd[:], ch12[:, 0:R], ch12[:, R:2 * R], AL.subtract)
                cw = gr.tile([128, R], I32, tag="cw")
                V.scalar_tensor_tensor(cw[:], ccd[:], 2.0, A1[:], AL.add, AL.is_lt)
                # Tsel = cw ? Mn*(2p2-1) : M*(2p1-1)
                i12 = gr.tile([128, 2 * R], F32, tag="i12")
                nc.scalar.activation(i12[:], p12f[:], AF.Copy, scale=2.0, bias=-1.0)
                tc12 = gr.tile([128, 2 * R], F32, tag="tc12")
                P.tensor_tensor(tc12[:], MMn[:], i12[:], AL.mult)
                t1c = tc12[:, 0:R]
                V.copy_predicated(t1c, cw[:], tc12[:, R:2 * R])  # Tsel
                # pm2 = 2-4*cw in {2,-2}; cwb = cw in {1,0} (bf16)
                pm2 = gr.tile([128, R], BF16, tag="pm2")
                nc.scalar.activation(pm2[:], cw[:], AF.Copy, scale=-4.0, bias=2.0)
                cwb = gr.tile([128, R], BF16, tag="cwb")
                nc.scalar.activation(cwb[:], cw[:], AF.Copy)

                # --- composition: sG = (2*mt*pm + cwb) * shalf = s*sigma ---
                mt = h16.tile([128, f], BF16, tag="mt")
                V.tensor_tensor(_g3(mt[:]), _g3(Ef), _bc(t1c), AL.is_equal)
                sa = h16.tile([128, f], BF16, tag="sa")
                h = f // 2
                P.tensor_tensor(_g3(sa[:, 0:h]), _g3(mt[:, 0:h]),
                                _bc(pm2[:, 0:R // 2]), AL.mult)
                V.tensor_tensor(_g3(sa[:, h:f]), _g3(mt[:, h:f]),
                                _bc(pm2[:, R // 2:R]), AL.mult)
                sg = h16.tile([128, f], BF16, tag="sg")
                q = 3 * f // 4
                P.tensor_tensor(_g3(sg[:, 0:q]), _g3(sa[:, 0:q]),
                                _bc(cwb[:, 0:3 * R // 4]), AL.add)
                V.tensor_tensor(_g3(sg[:, q:f]), _g3(sa[:, q:f]),
                                _bc(cwb[:, 3 * R // 4:R]), AL.add)
                sG = h16.tile([128, f], BF16, tag="sG")
                qq = 3 * f // 4
                P.tensor_tensor(sG[:, 0:qq], sg[:, 0:qq], shalf[:, 0:qq], AL.mult)
                V.tensor_tensor(sG[:, qq:f], sg[:, qq:f], shalf[:, qq:f], AL.mult)

                op_ = ps.tile([128, f], F32, tag="op_")
                for b0 in range(0, f, 512):
                    sl = slice(b0, b0 + 512)
                    nc.tensor.matmul(op_[:, sl], ident[:], f1[:, sl],
                                     start=True, stop=False)
                    nc.tensor.matmul(op_[:, sl], ident[:], sG[:, sl],
                                     start=False, stop=True)
                out = io.tile([128, f], F32, tag="out")
                nc.scalar.activation(out[:], op_[:], AF.Copy)
                nc.sync.dma_start(yt[t], out[:])

    if fix_multiwaits:
        _split_multiwaits(nc)
    return nc


_NC_CACHE = {}


def _get_nc(rows, f):
    key = (rows, f)
    if key not in _NC_CACHE:
        _NC_CACHE[key] = build_nc(rows, f)
    return _NC_CACHE[key]


def kernel(x: np.ndarray, _trace=False) -> np.ndarray:
    assert x.shape == (N_ROWS_FULL, DIM), x.shape
    x = np.ascontiguousarray(np.asarray(x, dtype=np.float32))
    nc = _get_nc(ROWS, F)
    in_maps = [
        {"x": np.ascontiguousarray(x[i * ROWS:(i + 1) * ROWS])}
        for i in range(NCORES)
    ]
    res = run_bass_kernel_spmd(nc, in_maps, core_ids=list(range(NCORES)),
                               trace=_trace)
    out = np.empty_like(x)
    for i in range(NCORES):
        out[i * ROWS:(i + 1) * ROWS] = res.results[i]["y"]
    return out


# revision 9
# speedup vs baseline: 1.1816x; 1.0024x over previous
"""Self-contained E8 lattice quantizer for Trainium2 (8 NeuronCores), v2.

kernel(x) -> nearest-E8-point of each row of x [8388608, 8] f32.

Algorithm (per group of 8 contiguous elements = one row):
  t1 = x + MAGIC ; f1 = t1 - MAGIC          (round-half-even via magic const)
  d1 = x - f1
  E  = (d1.bits & 0x7FFFFFF8) | (7 - idx)   (abs-bits + tie-break index)
  group stats via strided pairwise step + reduce4:
    M = max(E)  -> argmax|d1| encode        (coset-1 nudge target)
    Mn = min(E) -> argmin|d1| encode        (coset-2 nudge target)
    A1s = sum(E as f32) ~= sum|d1|
    Sf1 = sum(f1) bf16-exact, Ssh = sum(+-0.5 sign halves) -> neg count
  small [128,R] chain: parities via MAGIC-LSB, coset decision
    cw <=> par1*(2M-1) + 2 + 2*par2*Mn < A1s   (q2 < q1)
    Tsel = cw ? Mn*(2*par2-1) : M*(2*par1-1)   (negative => never matches)
  mt = (E == bc(Tsel)) as bf16; sG = (2*mt*bc(pm) + bc(cw)) * (+-0.5 by sign)
  out = f1 + sG via PE identity-matmul accumulate into PSUM (ACT evacuates)

Engine split balances ACT (rounding) / Pool / DVE per the v1 cost model.
Sharding: rows split evenly across 8 cores (data parallel, no comms).
"""
import numpy as np
import concourse.bass as bass
import concourse.mybir as mybir
from concourse.tile import TileContext
from concourse.bass_utils import run_bass_kernel_spmd

AL = mybir.AluOpType
AF = mybir.ActivationFunctionType
F32 = mybir.dt.float32
I32 = mybir.dt.int32
U16 = mybir.dt.uint16
BF16 = mybir.dt.bfloat16
MAGIC = float(np.float32(12582912.0))  # 1.5 * 2^23

N_ROWS_FULL = 8388608
DIM = 8
NCORES = 8
ROWS = N_ROWS_FULL // NCORES
F = 1024  # free-dim elems per partition per tile


def _split_multiwaits(nc):
    """This walrus build rejects >1 sem wait per instruction: hoist extras
    onto standalone nops inserted immediately before."""
    n = 0
    for f in nc.m.functions:
        for bb in f.blocks:
            newlist = []
            for ins in bb.instructions:
                si = getattr(ins, "sync_info", None)
                if si is not None and si.on_wait is not None and len(si.on_wait) > 1:
                    waits = list(si.on_wait)
                    for w in waits[:-1]:
                        nop = mybir.InstNoOp(name=f"I-mwfix-{n}", ins=[], outs=[])
                        n += 1
                        nop.engine = ins.engine
                        nop.sync_info = mybir.SyncInfo(on_wait=[w], on_update=[])
                        newlist.append(nop)
                    si.on_wait = [waits[-1]]
                newlist.append(ins)
            bb.instructions = newlist
    return n


def _g3(ap, c=8):
    return ap.rearrange("p (r c) -> p r c", c=c)


def _bc(ap_2d, c=8):
    p, r = ap_2d.shape
    return ap_2d.unsqueeze(2).broadcast_to((p, r, c))


def build_nc(rows=ROWS, f=F, num_devices=NCORES, fix_multiwaits=True):
    elems = rows * DIM
    assert elems % (128 * f) == 0
    ntiles = elems // (128 * f)
    R = f // 8

    nc = bass.Bass("TRN2", num_devices=num_devices, debug=False)
    x = nc.dram_tensor("x", [rows, DIM], F32, kind="ExternalInput")
    y = nc.dram_tensor("y", [rows, DIM], F32, kind="ExternalOutput")
    xt = x[:].flatten().rearrange("(t p f) -> t p f", p=128, f=f)
    yt = y[:].flatten().rearrange("(t p f) -> t p f", p=128, f=f)

    with TileContext(nc) as tc:
        with tc.tile_pool(name="cst", bufs=1) as cst, \
             tc.tile_pool(name="io", bufs=5) as io, \
             tc.tile_pool(name="wk", bufs=4) as wk, \
             tc.tile_pool(name="h16", bufs=4) as h16, \
             tc.tile_pool(name="g4", bufs=2) as g4, \
             tc.tile_pool(name="gr", bufs=3) as gr, \
             tc.tile_pool(name="ps", bufs=2, space="PSUM") as ps:

            # constant: (7 - idx%8) repeating along free dim
            idxf = cst.tile([128, f], I32)
            nc.gpsimd.iota(idxf[:], pattern=[[0, R], [1, 8]], base=0,
                           channel_multiplier=0)
            idxr = cst.tile([128, f], I32)
            nc.vector.tensor_scalar(idxr[:], idxf[:], -1, 7, AL.mult, AL.add)
            ii = cst.tile([128, 128], I32)
            nc.gpsimd.iota(ii[:], pattern=[[0, 128]], base=0, channel_multiplier=1)
            jj = cst.tile([128, 128], I32)
            nc.gpsimd.iota(jj[:], pattern=[[1, 128]], base=0, channel_multiplier=0)
            ident = cst.tile([128, 128], BF16)
            nc.vector.tensor_tensor(ident[:], ii[:], jj[:], AL.is_equal)
            mskA = cst.tile([128, 1], I32)
            nc.vector.memset(mskA[:], 0x7FFFFFF8)

            V, P = nc.vector, nc.gpsimd

            for t in range(ntiles):
                xv = io.tile([128, f], F32, tag="xv")
                nc.sync.dma_start(xv[:], xt[t])

                # rounding (ACT)
                t1 = wk.tile([128, f], F32, tag="t1")
                nc.scalar.activation(t1[:], xv[:], AF.Copy, bias=MAGIC)
                f1 = h16.tile([128, f], BF16, tag="f1")
                nc.scalar.activation(f1[:], t1[:], AF.Copy, bias=-MAGIC)

                # d1 (Pool)
                d1 = wk.tile([128, f], F32, tag="d1")
                P.tensor_tensor(d1[:], xv[:], f1[:], AL.subtract)
                d1i = d1[:].bitcast(I32)

                # E encode (DVE): (d1 & 0x7FFFFFF8) | idxr
                E = wk.tile([128, f], I32, tag="E")
                V.scalar_tensor_tensor(E[:], d1i, mskA[:, 0:1], idxr[:],
                                       AL.bitwise_and, AL.bitwise_or)
                Ef = E[:].bitcast(F32)

                # shalf = +-0.5 by sign of d1 (Pool, arith)
                shalf = h16.tile([128, f], BF16, tag="shalf")
                P.tensor_scalar(shalf[:], d1[:], 0.0, 0.5, AL.is_ge, AL.subtract)

                # --- group reductions ---
                MMn = gr.tile([128, 2 * R], F32, tag="MMn")

                def gtree(src_ap, op, tag, out_ap, e1, e2, e3, dt_mid=F32):
                    s4 = g4.tile([128, f // 2], dt_mid, tag=tag + "4")
                    a = _g3(src_ap)
                    e1.tensor_tensor(_g3(s4[:], 4), a[:, :, 0:4],
                                     a[:, :, 4:8], op)
                    s2 = g4.tile([128, f // 4], dt_mid, tag=tag + "2")
                    b = _g3(s4[:], 4)
                    e2.tensor_tensor(_g3(s2[:], 2), b[:, :, 0:2], b[:, :, 2:4], op)
                    c = _g3(s2[:], 2)
                    e3.tensor_tensor(out_ap.unsqueeze(2), c[:, :, 0:1],
                                     c[:, :, 1:2], op)

                gtree(Ef, AL.max, "M", MMn[:, 0:R], V, V, V)
                gtree(Ef, AL.min, "N", MMn[:, R:2 * R], V, V, V)
                A1 = gr.tile([128, R], F32, tag="A1")
                gtree(Ef, AL.add, "A", A1[:], P, P, P)
                Sf1 = gr.tile([128, R], F32, tag="C1")
                gtree(f1[:], AL.add, "C", Sf1[:], P, P, P, dt_mid=BF16)
                Ssh = gr.tile([128, R], F32, tag="S1")
                gtree(shalf[:], AL.add, "S", Ssh[:], P, P, P, dt_mid=BF16)

                # --- small-tile decision chain ---
                # pw2 halves: [C1+MAGIC | C1+MAGIC + (4-Ssh)]
                pw2 = gr.tile([128, 2 * R], F32, tag="pw2")
                nc.scalar.activation(pw2[:, 0:R], Sf1[:], AF.Copy, bias=MAGIC)
                Nn = gr.tile([128, R], F32, tag="Nn")
                nc.scalar.activation(Nn[:], Ssh[:], AF.Copy, scale=-1.0, bias=4.0)
                P.tensor_tensor(pw2[:, R:2 * R], pw2[:, 0:R], Nn[:], AL.add)
                p12 = gr.tile([128, 2 * R], I32, tag="p12")
                V.tensor_scalar(p12[:], pw2[:].bitcast(I32), 1, None, AL.bitwise_and)
                p12f = gr.tile([128, 2 * R], F32, tag="p12f")
                nc.scalar.activation(p12f[:], p12[:], AF.Copy)
                # mm12: [2M-1 | -2Mn]; ch12 = mm12*p12f = [c1 | c2]
                mm12 = gr.tile([128, 2 * R], F32, tag="mm12")
                nc.scalar.activation(mm12[:, 0:R], MMn[:, 0:R], AF.Copy, scale=2.0, bias=-1.0)
                nc.scalar.activation(mm12[:, R:2 * R], MMn[:, R:2 * R], AF.Copy, scale=-2.0)
                ch12 = gr.tile([128, 2 * R], F32, tag="ch12")
                P.tensor_tensor(ch12[:], mm12[:], p12f[:], AL.mult)
                ccd = gr.tile([128, R], F32, tag="ccd")
                P.tensor_tensor(ccd[:], ch12[:, 0:R], ch12[:, R:2 * R], AL.subtract)
                cw = gr.tile([128, R], I32, tag="cw")
                V.scalar_tensor_tensor(cw[:], ccd[:], 2.0, A1[:], AL.add, AL.is_lt)
                # Tsel = cw ? Mn*(2p2-1) : M*(2p1-1)
                i12 = gr.tile([128, 2 * R], F32, tag="i12")
                nc.scalar.activation(i12[:], p12f[:], AF.Copy, scale=2.0, bias=-1.0)
                tc12 = gr.tile([128, 2 * R], F32, tag="tc12")
                P.tensor_tensor(tc12[:], MMn[:], i12[:], AL.mult)
                t1c = tc12[:, 0:R]
                V.copy_predicated(t1c, cw[:], tc12[:, R:2 * R])  # Tsel
                # pm2 = 2-4*cw in {2,-2}; cwb = cw in {1,0} (bf16)
                pm2 = gr.tile([128, R], BF16, tag="pm2")
                nc.scalar.activation(pm2[:], cw[:], AF.Copy, scale=-4.0, bias=2.0)
                cwb = gr.tile([128, R], BF16, tag="cwb")
                nc.scalar.activation(cwb[:], cw[:], AF.Copy)

                # --- composition: sG = (2*mt*pm + cwb) * shalf = s*sigma ---
                mt = h16.tile([128, f], BF16, tag="mt")
                V.tensor_tensor(_g3(mt[:]), _g3(Ef), _bc(t1c), AL.is_equal)
                sa = h16.tile([128, f], BF16, tag="sa")
                h = f // 2
                P.tensor_tensor(_g3(sa[:, 0:h]), _g3(mt[:, 0:h]),
                                _bc(pm2[:, 0:R // 2]), AL.mult)
                V.tensor_tensor(_g3(sa[:, h:f]), _g3(mt[:, h:f]),
                                _bc(pm2[:, R // 2:R]), AL.mult)
                sg = h16.tile([128, f], BF16, tag="sg")
                q = 3 * f // 4
                P.tensor_tensor(_g3(sg[:, 0:q]), _g3(sa[:, 0:q]),
                                _bc(cwb[:, 0:3 * R // 4]), AL.add)
                V.tensor_tensor(_g3(sg[:, q:f]), _g3(sa[:, q:f]),
                                _bc(cwb[:, 3 * R // 4:R]), AL.add)
                sG = h16.tile([128, f], BF16, tag="sG")
                qq = 3 * f // 4
                P.tensor_tensor(sG[:, 0:qq], sg[:, 0:qq], shalf[:, 0:qq], AL.mult)
                V.tensor_tensor(sG[:, qq:f], sg[:, qq:f], shalf[:, qq:f], AL.mult)

                op_ = ps.tile([128, f], F32, tag="op_")
                for b0 in range(0, f, 512):
                    sl = slice(b0, b0 + 512)
                    nc.tensor.matmul(op_[:, sl], ident[:], f1[:, sl],
                                     start=True, stop=False)
                    nc.tensor.matmul(op_[:, sl], ident[:], sG[:, sl],
                                     start=False, stop=True)
                out = io.tile([128, f], F32, tag="out")
                hh = f // 2
                nc.scalar.activation(out[:, 0:hh], op_[:, 0:hh], AF.Copy)
                nc.sync.dma_start(yt[t][:, 0:hh], out[:, 0:hh])
                nc.scalar.activation(out[:, hh:f], op_[:, hh:f], AF.Copy)
                nc.sync.dma_start(yt[t][:, hh:f], out[:, hh:f])

    if fix_multiwaits:
        _split_multiwaits(nc)
    return nc


_NC_CACHE = {}


def _get_nc(rows, f):
    key = (rows, f)
    if key not in _NC_CACHE:
        _NC_CACHE[key] = build_nc(rows, f)
    return _NC_CACHE[key]


def kernel(x: np.ndarray, _trace=False) -> np.ndarray:
    assert x.shape == (N_ROWS_FULL, DIM), x.shape
    x = np.ascontiguousarray(np.asarray(x, dtype=np.float32))
    nc = _get_nc(ROWS, F)
    in_maps = [
        {"x": np.ascontiguousarray(x[i * ROWS:(i + 1) * ROWS])}
        for i in range(NCORES)
    ]
    res = run_bass_kernel_spmd(nc, in_maps, core_ids=list(range(NCORES)),
                               trace=_trace)
    out = np.empty_like(x)
    for i in range(NCORES):
        out[i * ROWS:(i + 1) * ROWS] = res.results[i]["y"]
    return out
